# revision 13
# baseline (speedup 1.0000x reference)
"""Trainium2 Bass kernel: nearest-centroid assignment (vq_codebook).

For each row of `feats` [N, 512]:
    f = normalize([feats_n, 1])            (L2, with appended ones column)
    pred_n = labelset[argmin_l ||f - c_l||]   with c = initc[labelset]  [128, 513]

Equivalent argmax formulation used on device (monotone transform, per row n):
    argmin_l dist  ==  argmax_l  T[n, l]
    T[n, l] = sum_d feats[n,d] * c[l,d]  +  cdvh_l + cdvl_l  +  u_n * (-w2c_l)
    u_n  = sqrt(|feats_n|^2 + 1) - sqrt(513)
    w2c_l = 0.5*|c_l|^2 - mean_l(0.5*|c_l|^2);  cdv_l = c[l,512] - sqrt(513)*w2c_l
    (cdv ships as bf16-exact hi + fp16 lo so every fp16 operand is small or exact)

Sharding: pure data-parallel over rows, N/8 = 32768 rows per NeuronCore.

Per-core dataflow (groups of GROUP=1024 rows = 8 subtiles of 128):
    DMA (gpsimd SWDGE): f32 -> fp16 in-flight, 4 consecutive rows per
        partition => 8 KiB contiguous HBM reads per descriptor
    DVE/ACT/Pool: square+accum -> q per row; ACT sqrt -> r; DVE writes
        u = r - sqrt(513) into a tiny side tile Fgx[:, :, 2] whose cols
        0..1 are memset to 1.0 (the "virtual dims" [1, 1, u])
    PE: 32x transpose 128x128 fp16 -> fT (d on partitions), 8x transpose
        [128, 3] Fgx -> [3, 128] mini-chunks
    DVE/ACT: copy fT PSUM->SBUF (fp16)
    PE: T[n, l] accumulation with ft as the STATIONARY operand:
        32 matmuls (128-free) + 8 rank-1 matmuls [1,1,u]x[cdvh,cdvl,-w2c]
        (128-free); all fp16 into one [128, 8, 128] f32 PSUM tile
    DVE: pool_max over the 8 windows of 128 -> [128, 8] window maxima,
         one max_index over all 1024 scores -> global positions (u16),
         written straight into the output accumulator
    final single DMA of idx [128, ngroups, 8] u16; host decodes l = idx & 127

Scores stay exact fp32 from PSUM accumulation through pool_max/max_index
(both read PSUM directly), so the only fp16 roundings are the feats cast,
u, and -w2c - same error budget as the 313us baseline (measured 1/262144
label mismatches), but with ~2x less DVE/ACT/PE work per row.

Notes on this walrus build:
  - one sync wait per engine instruction: constants ship in single-DMA packs
    + PE/ACT warmup ops observe them; residual multi-wait instructions are
    split via same-engine NoOps (_split_multiwait).
  - all matmuls accumulating into one PSUM tile are fp16 (never mix f32r).
"""

import os
import sys

import numpy as np

for _p in ("/opt/trn_rl_repo",):
    if _p not in sys.path and os.path.isdir(_p):
        sys.path.insert(0, _p)

import concourse.bass as bass
import concourse.mybir as mybir
import concourse.tile as tile
from concourse.bass_utils import run_bass_kernel_spmd

N, D, K = 262144, 512, 128
N_CORES = 8
ROWS_PER_CORE = N // N_CORES  # 32768
GROUP = 1024  # rows per group (8 subtiles of 128)

F32 = mybir.dt.float32
F16 = mybir.dt.float16
U16 = mybir.dt.uint16
AF = mybir.ActivationFunctionType
ALU = mybir.AluOpType

# cpk16 layout (columns of one [128, 768] fp16 constant block, single DMA)
P16_IDENT = 0    # [128, 128] identity
P16_CT = 128     # [128, 512] cT: cT[p, 128k+l] = c[l, 128k+p]
P16_WCD = 640    # [3, 128]: row0 cdvh, row1 cdvl, row2 -w2c
P16_W = 768

MBAR = float(np.sqrt(513.0))  # fixed shift for r; argmax-invariant


def _split_multiwait(nc):
    """Walrus (this build) allows one sync wait per engine instruction.

    Tile occasionally emits 2+ (data dep + buffer-slot release on another
    semaphore). Splitting is semantics-preserving: a same-engine NoOp placed
    immediately before the instruction carries the surplus waits; the engine
    executes in order, so all waits are still enforced before the instruction
    runs.
    """
    import bass_rust

    for fn in nc.m.functions:
        for blk in fn.blocks:
            out = []
            changed = False
            for inst in blk.instructions:
                si = getattr(inst, "sync_info", None)
                waits = list(si.on_wait) if si is not None else []
                if len(waits) > 1:
                    for w in waits[:-1]:
                        nop = mybir.InstNoOp(
                            name=nc.get_next_instruction_name(), ins=[], outs=[]
                        )
                        nop.engine = inst.engine
                        nop.sync_info = bass_rust.SyncInfo(
                            on_wait=[w], on_update=[]
                        )
                        out.append(nop)
                    inst.sync_info = bass_rust.SyncInfo(
                        on_wait=waits[-1:], on_update=list(si.on_update)
                    )
                    changed = True
                out.append(inst)
            if changed:
                blk.instructions = out


def build_core_program(
    rows=ROWS_PER_CORE,
    split_waits=True,
    psum_argmax=True,   # tensor_reduce/max_index read PSUM directly
    new_dma=True,       # 4-consecutive-rows-per-partition (8KB reads)
    minis=True,         # rank-1 fixups via [128,3] mini transposes
):
    """Bass program for one NeuronCore processing `rows` rows of feats."""
    assert rows % GROUP == 0
    ngroups = rows // GROUP

    nc = bass.Bass()
    feats_d = nc.declare_dram_parameter("feats", [rows, D], F32, isOutput=False)
    cpk16_d = nc.declare_dram_parameter("cpk16", [128, P16_W], F16, isOutput=False)
    cb32_d = nc.declare_dram_parameter("cb32", [128, 2], F32, isOutput=False)
    idx_d = nc.declare_dram_parameter("idx16", [128, ngroups, 8], U16, isOutput=True)

    with tile.TileContext(nc) as tc:
        with (
            tc.tile_pool(name="const", bufs=1) as constp,
            tc.tile_pool(name="fin", bufs=3) as finp,
            tc.tile_pool(name="fgx", bufs=2) as fgxp,
            tc.tile_pool(name="sq", bufs=4) as sqp,
            tc.tile_pool(name="qr", bufs=2) as qrp,
            tc.tile_pool(name="ftsb", bufs=4) as ftsbp,
            tc.tile_pool(name="ftm", bufs=2) as ftmsbp,
            tc.tile_pool(name="pm", bufs=2) as pmp,
            tc.tile_pool(name="outp", bufs=1) as outp,
            tc.tile_pool(name="ftp", bufs=2, space="PSUM") as ftpp,
            tc.tile_pool(name="tp", bufs=2, space="PSUM") as tpp,
            tc.tile_pool(name="ftmp", bufs=2, space="PSUM") as ftmpp,
        ):
            cpk16 = constp.tile([128, P16_W], F16)
            nc.sync.dma_start(cpk16[:], cpk16_d[:])
            ident16 = cpk16[:, P16_IDENT : P16_IDENT + 128]
            cT16 = cpk16[:, P16_CT : P16_CT + 512]
            wcd16 = cpk16[0:3, P16_WCD : P16_WCD + 128]
            cb32 = constp.tile([128, 2], F32)
            nc.sync.dma_start(cb32[:], cb32_d[:])
            bias1 = cb32[:, 0:1]
            bias0 = cb32[:, 1:2]
            idxacc = outp.tile([128, ngroups, 8], U16)

            # warmup: make PE/ACT observe the const DMA lanes via single-wait
            # instructions so later ops carry at most one new wait each.
            warm_ps = ftmpp.tile([128, 128], F16, tag="ftm_ps")
            nc.tensor.transpose(warm_ps[:], ident16, ident16)
            act_warm = qrp.tile([1, 1], F32, tag="act_warm")
            nc.scalar.copy(act_warm[:], cb32[0:1, 0:1])
            if not minis:
                ftm0 = constp.tile([3, 8, 128], F16)
                nc.vector.memset(ftm0[:], 0.0)

            for g in range(ngroups):
                # feats DMA: Fg[p, 4h+j, d] = feats[g*1024 + 512h + 4p + j, d]
                # (4 consecutive rows per partition => 8 KiB HBM reads)
                Fg = finp.tile([128, 8, 512], F16)
                if new_dma:
                    src = feats_d[g * GROUP : (g + 1) * GROUP, :].rearrange(
                        "(h p j) d -> p h j d", h=2, p=128
                    )
                    dst = Fg[:].rearrange("p (h j) d -> p h j d", h=2)
                    nc.gpsimd.dma_start(dst, src)
                else:
                    src = feats_d[g * GROUP : (g + 1) * GROUP, :].rearrange(
                        "(j p) d -> p j d", p=128
                    )
                    nc.gpsimd.dma_start(Fg[:], src)

                # q_w = |row|^2 per partition (walrus allows no Pool compute:
                # all squares ride DVE, all copies ride ACT)
                Q = qrp.tile([128, 8], F32, tag="Q")
                for w in range(8):
                    sq = sqp.tile([128, 512], F16, tag="sq")
                    nc.vector.scalar_tensor_tensor(
                        sq[:], Fg[:, w, :], 1.0, Fg[:, w, :],
                        ALU.mult, ALU.mult, accum_out=Q[:, w : w + 1],
                    )
                R = qrp.tile([128, 8], F32, tag="R")
                nc.scalar.activation(R[:], Q[:], AF.Sqrt, bias=bias1, scale=1.0)

                # virtual dims [1, 1, u] per row (u = r - MBAR, fp16)
                if minis:
                    Fgx = fgxp.tile([128, 8, 3], F16)
                    nc.vector.memset(Fgx[:, :, 0:2], 1.0)
                    nc.vector.tensor_scalar_add(
                        Fgx[:, :, 2:3], R[:].unsqueeze(2), -MBAR
                    )

                # T[n, l] accumulation: per k-chunk transpose Fg -> ft, then
                # 8 matmuls with ft as the stationary operand.
                T_ps = tpp.tile([128, 8, 128], F32)
                for k in range(4):
                    ft_ps = ftpp.tile([128, 1024], F16)
                    for j in range(8):
                        nc.tensor.transpose(
                            ft_ps[:, j * 128 : (j + 1) * 128],
                            Fg[:, j, k * 128 : (k + 1) * 128],
                            ident16,
                        )
                    ft = ftsbp.tile([128, 1024], F16)
                    nc.scalar.copy(ft[:], ft_ps[:])
                    for j in range(8):
                        # PSUM zero regions are bank-granular (4 slices):
                        # start once per bank, the bank-mates ride on it.
                        nc.tensor.matmul(
                            T_ps[:, j, :],
                            ft[:, j * 128 : (j + 1) * 128],
                            cT16[:, k * 128 : (k + 1) * 128],
                            start=(k == 0 and j % 4 == 0),
                            stop=False,
                        )

                # rank-1 fixups: [3,128] mini transposes + matmuls vs wcd16
                if minis:
                    ftm_ps = ftmpp.tile([3, 8, 128], F16, tag="ftm_ps")
                    for j in range(8):
                        nc.tensor.transpose(
                            ftm_ps[:, j, :], Fgx[:, j, :], ident16
                        )
                    ftm = ftmsbp.tile([3, 8, 128], F16)
                    nc.scalar.copy(ftm[:], ftm_ps[:])
                    for j in range(8):
                        nc.tensor.matmul(
                            T_ps[:, j, :], ftm[:, j, :], wcd16,
                            start=False, stop=(j % 4 == 3),
                        )
                else:
                    for j in range(8):
                        nc.tensor.matmul(
                            T_ps[:, j, :], ftm0[:, j, :], wcd16,
                            start=False, stop=(j % 4 == 3),
                        )

                # argmax over l per row: window maxima + one global max_index
                if psum_argmax:
                    T_rd = T_ps
                else:
                    T_rd = pmp.tile([128, 8, 128], F32, tag="st")
                    nc.scalar.copy(T_rd[:], T_ps[:])
                pm8 = pmp.tile([128, 8], F32)
                nc.vector.tensor_reduce(
                    pm8[:], T_rd[:], mybir.AxisListType.X, ALU.max,
                    opt_input=False,
                )
                nc.vector.max_index(
                    idxacc[:, g, :], pm8[:],
                    T_rd[:].rearrange("p j l -> p (j l)"),
                )

            nc.sync.dma_start(idx_d[:], idxacc[:])
    if split_waits:
        _split_multiwait(nc)
    return nc


def make_const_inputs(initc, labelset):
    c = np.asarray(initc, dtype=np.float32)[np.asarray(labelset).astype(np.int64)]
    assert c.shape == (K, D + 1)
    w2 = 0.5 * np.sum(c.astype(np.float64) ** 2, axis=1)
    # Centering w2 and r shifts scores by per-row constants (argmax invariant)
    # while keeping the fp16-resident rank-1 operands small. The leftover
    # per-l constant cdv = cd - MBAR*w2c is large, so it ships as a
    # bf16-exact hi part (fp16-exact too) plus a small fp16 lo remainder.
    w2c = w2 - w2.mean()
    cdv = c[:, D].astype(np.float64) - MBAR * w2c
    cdv_hi = cdv.astype(np.float32)
    cdv_hi = (cdv_hi.view(np.uint32) & np.uint32(0xFFFF0000)).view(np.float32)
    cdv_lo = (cdv - cdv_hi.astype(np.float64)).astype(np.float32)
    cdv_hi16 = cdv_hi.astype(np.float16)
    assert np.array_equal(cdv_hi16.astype(np.float32), cdv_hi), "cdv_hi not fp16-exact"

    cpk16 = np.zeros((128, P16_W), np.float16)
    cpk16[:, P16_IDENT : P16_IDENT + 128] = np.eye(128, dtype=np.float16)
    for k in range(4):
        cpk16[:, P16_CT + k * 128 : P16_CT + (k + 1) * 128] = (
            c[:, k * 128 : (k + 1) * 128].T.astype(np.float16)
        )
    cpk16[0, P16_WCD : P16_WCD + 128] = cdv_hi16
    cpk16[1, P16_WCD : P16_WCD + 128] = cdv_lo.astype(np.float16)
    cpk16[2, P16_WCD : P16_WCD + 128] = (-w2c).astype(np.float16)

    cb32 = np.zeros((128, 2), np.float32)
    cb32[:, 0] = 1.0
    return {"cpk16": cpk16, "cb32": cb32}


def decode_idx(idx16, labelset_np):
    """[128, ngroups, 8] u16 window-argmax positions -> labels in row order."""
    ngroups = idx16.shape[1]
    l = (idx16.astype(np.int64) & 127)  # [p, g, w]
    # row n = g*1024 + (w//4)*512 + 4p + (w%4)
    l = l.reshape(128, ngroups, 2, 4)           # [p, g, h, j]
    l = np.transpose(l, (1, 2, 0, 3)).reshape(-1)  # [g, h, p, j] flattened
    return np.asarray(labelset_np)[l]


def kernel(feats, initc, labelset):
    feats = np.asarray(feats, dtype=np.float32)
    labelset_np = np.asarray(labelset)
    consts = make_const_inputs(initc, labelset)

    nc = build_core_program(ROWS_PER_CORE)
    in_maps = []
    for core in range(N_CORES):
        shard = feats[core * ROWS_PER_CORE : (core + 1) * ROWS_PER_CORE]
        in_maps.append({"feats": np.ascontiguousarray(shard), **consts})
    res = run_bass_kernel_spmd(nc, in_maps, list(range(N_CORES)))

    preds = []
    for core in range(N_CORES):
        idx16 = np.asarray(res.results[core]["idx16"])  # [128, ngroups, 8]
        preds.append(decode_idx(idx16, labelset_np))
    return np.concatenate(preds)


# revision 19
# speedup vs baseline: 1.5439x; 1.5439x over previous
"""Trainium2 Bass kernel: nearest-centroid assignment (vq_codebook).

For each row of `feats` [N, 512]:
    f = normalize([feats_n, 1])            (L2, with appended ones column)
    pred_n = labelset[argmin_l ||f - c_l||]   with c = initc[labelset]  [128, 513]

Equivalent argmax formulation used on device (monotone transform, per row n):
    argmin_l dist  ==  argmax_l  T[n, l]
    T[n, l] = sum_d feats[n,d] * c[l,d]  +  cdvh_l + cdvl_l  +  u_n * (-w2c_l)
    u_n  = sqrt(|feats_n|^2 + 1) - sqrt(513)
    w2c_l = 0.5*|c_l|^2 - mean_l(0.5*|c_l|^2);  cdv_l = c[l,512] - sqrt(513)*w2c_l
    (cdv ships as bf16-exact hi + fp16 lo so every fp16 operand is small or exact)

Sharding: pure data-parallel over rows, N/8 = 32768 rows per NeuronCore.

Per-core dataflow (groups of GROUP=1024 rows = 8 subtiles of 128):
    DMA (gpsimd SWDGE): f32 -> fp16 in-flight, 4 consecutive rows per
        partition => 8 KiB contiguous HBM reads per descriptor
    DVE/ACT: square+accum -> q per row (split across both engines); ACT
        sqrt -> r; DVE writes u = r - sqrt(513) into Fgx[:, 2, :] whose
        rows 0..1 are memset to 1.0 (the "virtual dims" [1, 1, u])
    PE: 32x transpose 128x128 fp16 -> fT (d on partitions); 8x transpose
        [128, 3] Fgx -> [3, 128] mini-chunks placed at 32-aligned
        partition offsets so the rank-1 matmuls stay legal
    DVE: copy fT PSUM->SBUF (fp16, 2x mode), tiny [128, 2, 128] mini copy
    PE: T[n, l] accumulation with ft as the STATIONARY operand:
        32 matmuls (128-free) + 8 rank-1 matmuls [1,1,u]x[cdvh,cdvl,-w2c]
        into one [128, 8, 128] f32 PSUM tile
    DVE: tensor_reduce max over the 8 windows of 128 -> [128, 8] maxima,
         one max_index over all 1024 scores -> positions (u16) straight
         into the output accumulator; final single DMA; host: l = idx & 127

Scores stay exact fp32 from PSUM accumulation through reduce/max_index
(both read PSUM directly), so the only fp16 roundings are the feats cast,
u, and -w2c - same error budget as the 313us baseline (measured 1/262144
label mismatches).

Notes on this walrus build:
  - one sync wait per engine instruction: constants ship in single-DMA packs
    + PE/ACT warmup ops observe them; residual multi-wait instructions are
    split via same-engine NoOps (_split_multiwait).
  - no Pool-engine compute (walrus rejects it); Pool only issues the DMA.
"""

import os
import sys

import numpy as np

for _p in ("/opt/trn_rl_repo",):
    if _p not in sys.path and os.path.isdir(_p):
        sys.path.insert(0, _p)

import concourse.bass as bass
import concourse.mybir as mybir
import concourse.tile as tile
from concourse.bass_utils import run_bass_kernel_spmd

N, D, K = 262144, 512, 128
N_CORES = 8
ROWS_PER_CORE = N // N_CORES  # 32768
GROUP = 1024  # rows per group (8 subtiles of 128)
SQ_DVE_N = 1  # squares on DVE (rest on ACT)

F32 = mybir.dt.float32
F16 = mybir.dt.float16
U16 = mybir.dt.uint16
AF = mybir.ActivationFunctionType
ALU = mybir.AluOpType

# cpk16 layout (columns of one [128, 768] fp16 constant block, single DMA)
P16_IDENT = 0    # [128, 128] identity
P16_CT = 128     # [128, 512] cT: cT[p, 128k+l] = c[l, 128k+p]
P16_WCD = 640    # [3, 128] at partition offsets 0/32/64/96:
                 #   row0 cdvh, row1 cdvl, row2 -w2c (replicated 4x)
P16_W = 768

MBAR = float(np.sqrt(513.0))  # fixed shift for r; argmax-invariant


def _split_multiwait(nc):
    """Walrus (this build) allows one sync wait per engine instruction.

    Tile occasionally emits 2+ (data dep + buffer-slot release on another
    semaphore). Splitting is semantics-preserving: a same-engine NoOp placed
    immediately before the instruction carries the surplus waits; the engine
    executes in order, so all waits are still enforced before the instruction
    runs.
    """
    import bass_rust

    for fn in nc.m.functions:
        for blk in fn.blocks:
            out = []
            changed = False
            for inst in blk.instructions:
                si = getattr(inst, "sync_info", None)
                waits = list(si.on_wait) if si is not None else []
                if len(waits) > 1:
                    for w in waits[:-1]:
                        nop = mybir.InstNoOp(
                            name=nc.get_next_instruction_name(), ins=[], outs=[]
                        )
                        nop.engine = inst.engine
                        nop.sync_info = bass_rust.SyncInfo(
                            on_wait=[w], on_update=[]
                        )
                        out.append(nop)
                    inst.sync_info = bass_rust.SyncInfo(
                        on_wait=waits[-1:], on_update=list(si.on_update)
                    )
                    changed = True
                out.append(inst)
            if changed:
                blk.instructions = out


def build_core_program(rows=ROWS_PER_CORE, split_waits=True):
    """Bass program for one NeuronCore processing `rows` rows of feats."""
    assert rows % GROUP == 0
    ngroups = rows // GROUP

    nc = bass.Bass()
    feats_d = nc.declare_dram_parameter("feats", [rows, D], F32, isOutput=False)
    cpk16_d = nc.declare_dram_parameter("cpk16", [128, P16_W], F16, isOutput=False)
    cb32_d = nc.declare_dram_parameter("cb32", [128, 2], F32, isOutput=False)
    idx_d = nc.declare_dram_parameter("idx16", [128, ngroups, 8], U16, isOutput=True)

    with tile.TileContext(nc) as tc:
        with (
            tc.tile_pool(name="const", bufs=1) as constp,
            tc.tile_pool(name="fin", bufs=3) as finp,
            tc.tile_pool(name="fgx", bufs=2) as fgxp,
            tc.tile_pool(name="sq", bufs=4) as sqp,
            tc.tile_pool(name="qr", bufs=2) as qrp,
            tc.tile_pool(name="ftsb", bufs=4) as ftsbp,
            tc.tile_pool(name="ftm", bufs=2) as ftmsbp,
            tc.tile_pool(name="pm", bufs=2) as pmp,
            tc.tile_pool(name="outp", bufs=1) as outp,
            tc.tile_pool(name="ftp", bufs=3, space="PSUM") as ftpp,
            tc.tile_pool(name="tp", bufs=2, space="PSUM") as tpp,
            tc.tile_pool(name="ftmp", bufs=1, space="PSUM") as ftmpp,
        ):
            cpk16 = constp.tile([128, P16_W], F16)
            nc.sync.dma_start(cpk16[:], cpk16_d[:])
            ident16 = cpk16[:, P16_IDENT : P16_IDENT + 128]
            cT16 = cpk16[:, P16_CT : P16_CT + 512]
            cb32 = constp.tile([128, 2], F32)
            nc.sync.dma_start(cb32[:], cb32_d[:])
            bias1 = cb32[:, 0:1]
            bias0 = cb32[:, 1:2]
            idxacc = outp.tile([128, ngroups, 8], U16)

            # warmup: make PE/ACT observe the const DMA lanes via single-wait
            # instructions so later ops carry at most one new wait each.
            warm_ps = ftmpp.tile([3, 8, 128], F16, tag="ftm_ps")
            nc.tensor.transpose(warm_ps[:, 0, :], ident16[:, 0:3], ident16)
            act_warm = qrp.tile([1, 1], F32, tag="act_warm")
            nc.scalar.copy(act_warm[:], cb32[0:1, 0:1])

            for g in range(ngroups):
                # feats DMA: Fg[p, 4h+j, d] = feats[g*1024 + 512h + 4p + j, d]
                # (4 consecutive rows per partition => 8 KiB HBM reads)
                Fg = finp.tile([128, 8, 512], F16)
                src = feats_d[g * GROUP : (g + 1) * GROUP, :].rearrange(
                    "(h p j) d -> p h j d", h=2, p=128
                )
                dst = Fg[:].rearrange("p (h j) d -> p h j d", h=2)
                nc.gpsimd.dma_start(dst, src)

                # q_w = |row|^2 per partition; split DVE / ACT
                Q = qrp.tile([128, 8], F32, tag="Q")
                for w in range(8):
                    sq = sqp.tile([128, 512], F16, tag="sq")
                    if w < SQ_DVE_N:
                        nc.vector.scalar_tensor_tensor(
                            sq[:], Fg[:, w, :], 1.0, Fg[:, w, :],
                            ALU.mult, ALU.mult, accum_out=Q[:, w : w + 1],
                        )
                    else:
                        nc.scalar.activation(
                            sq[:], Fg[:, w, :], AF.Square,
                            bias=bias0, accum_out=Q[:, w : w + 1],
                        )
                R = qrp.tile([128, 8], F32, tag="R")
                nc.scalar.activation(R[:], Q[:], AF.Sqrt, bias=bias1, scale=1.0)

                # virtual dims [1, 1, u] per row (u = r - MBAR, fp16);
                # contiguous [128, 3, 8] layout keeps the writes cheap
                Fgx = fgxp.tile([128, 3, 8], F16)
                nc.vector.memset(Fgx[:, 0:2, :], 1.0)
                nc.vector.tensor_scalar_add(Fgx[:, 2, :], R[:], -MBAR)

                # T[n, l] accumulation: per k-chunk transpose Fg -> ft, then
                # 8 matmuls with ft as the stationary operand.
                T_ps = tpp.tile([128, 8, 128], F32)
                for k in range(4):
                    ft_ps = ftpp.tile([128, 1024], F16)
                    for j in range(8):
                        nc.tensor.transpose(
                            ft_ps[:, j * 128 : (j + 1) * 128],
                            Fg[:, j, k * 128 : (k + 1) * 128],
                            ident16,
                        )
                    ft = ftsbp.tile([128, 1024], F16)
                    nc.vector.tensor_copy(ft[:], ft_ps[:])
                    for j in range(8):
                        # PSUM zero regions are bank-granular (4 slices):
                        # start once per bank, the bank-mates ride on it.
                        nc.tensor.matmul(
                            T_ps[:, j, :],
                            ft[:, j * 128 : (j + 1) * 128],
                            cT16[:, k * 128 : (k + 1) * 128],
                            start=(k == 0 and j % 4 == 0),
                            stop=False,
                        )

                # rank-1 fixups: [128, 3] mini transposes (each j gets its
                # own PSUM free-range - zero regions ignore partitions), one
                # [3, 8, 128] copy on DVE, then 8 contract-3 matmuls vs wcd16.
                ftm_ps = ftmpp.tile([3, 8, 128], F16, tag="ftm_ps")
                for j in range(8):
                    nc.tensor.transpose(ftm_ps[:, j, :], Fgx[:, :, j], ident16)
                ftm = ftmsbp.tile([3, 8, 128], F16)
                nc.vector.tensor_copy(ftm[:], ftm_ps[:])
                for j in range(8):
                    nc.tensor.matmul(
                        T_ps[:, j, :],
                        ftm[:, j, :],
                        cpk16[0:3, P16_WCD : P16_WCD + 128],
                        start=False, stop=(j % 4 == 3),
                    )

                # argmax over l per row: window maxima + one global max_index,
                # both reading the f32 scores straight from PSUM
                pm8 = pmp.tile([128, 8], F32)
                nc.vector.tensor_reduce(
                    pm8[:], T_ps[:], mybir.AxisListType.X, ALU.max,
                    opt_input=False,
                )
                nc.vector.max_index(
                    idxacc[:, g, :], pm8[:],
                    T_ps[:].rearrange("p j l -> p (j l)"),
                )

            nc.sync.dma_start(idx_d[:], idxacc[:])
    if split_waits:
        _split_multiwait(nc)
    return nc


def make_const_inputs(initc, labelset):
    c = np.asarray(initc, dtype=np.float32)[np.asarray(labelset).astype(np.int64)]
    assert c.shape == (K, D + 1)
    w2 = 0.5 * np.sum(c.astype(np.float64) ** 2, axis=1)
    # Centering w2 and r shifts scores by per-row constants (argmax invariant)
    # while keeping the fp16-resident rank-1 operands small. The leftover
    # per-l constant cdv = cd - MBAR*w2c is large, so it ships as a
    # bf16-exact hi part (fp16-exact too) plus a small fp16 lo remainder.
    w2c = w2 - w2.mean()
    cdv = c[:, D].astype(np.float64) - MBAR * w2c
    cdv_hi = cdv.astype(np.float32)
    cdv_hi = (cdv_hi.view(np.uint32) & np.uint32(0xFFFF0000)).view(np.float32)
    cdv_lo = (cdv - cdv_hi.astype(np.float64)).astype(np.float32)
    cdv_hi16 = cdv_hi.astype(np.float16)
    assert np.array_equal(cdv_hi16.astype(np.float32), cdv_hi), "cdv_hi not fp16-exact"

    cpk16 = np.zeros((128, P16_W), np.float16)
    cpk16[:, P16_IDENT : P16_IDENT + 128] = np.eye(128, dtype=np.float16)
    for k in range(4):
        cpk16[:, P16_CT + k * 128 : P16_CT + (k + 1) * 128] = (
            c[:, k * 128 : (k + 1) * 128].T.astype(np.float16)
        )
    for q in range(3):
        cpk16[32 * q + 0, P16_WCD : P16_WCD + 128] = cdv_hi16
        cpk16[32 * q + 1, P16_WCD : P16_WCD + 128] = cdv_lo.astype(np.float16)
        cpk16[32 * q + 2, P16_WCD : P16_WCD + 128] = (-w2c).astype(np.float16)

    cb32 = np.zeros((128, 2), np.float32)
    cb32[:, 0] = 1.0
    return {"cpk16": cpk16, "cb32": cb32}


def decode_idx(idx16, labelset_np):
    """[128, ngroups, 8] u16 window-argmax positions -> labels in row order."""
    ngroups = idx16.shape[1]
    l = (idx16.astype(np.int64) & 127)  # [p, g, w]
    # row n = g*1024 + (w//4)*512 + 4p + (w%4)
    l = l.reshape(128, ngroups, 2, 4)           # [p, g, h, j]
    l = np.transpose(l, (1, 2, 0, 3)).reshape(-1)  # [g, h, p, j] flattened
    return np.asarray(labelset_np)[l]


def kernel(feats, initc, labelset):
    feats = np.asarray(feats, dtype=np.float32)
    labelset_np = np.asarray(labelset)
    consts = make_const_inputs(initc, labelset)

    nc = build_core_program(ROWS_PER_CORE)
    in_maps = []
    for core in range(N_CORES):
        shard = feats[core * ROWS_PER_CORE : (core + 1) * ROWS_PER_CORE]
        in_maps.append({"feats": np.ascontiguousarray(shard), **consts})
    res = run_bass_kernel_spmd(nc, in_maps, list(range(N_CORES)))

    preds = []
    for core in range(N_CORES):
        idx16 = np.asarray(res.results[core]["idx16"])  # [128, ngroups, 8]
        preds.append(decode_idx(idx16, labelset_np))
    return np.concatenate(preds)


# revision 24
# speedup vs baseline: 1.5696x; 1.0166x over previous
"""Trainium2 Bass kernel: nearest-centroid assignment (vq_codebook).

For each row of `feats` [N, 512]:
    f = normalize([feats_n, 1])            (L2, with appended ones column)
    pred_n = labelset[argmin_l ||f - c_l||]   with c = initc[labelset]  [128, 513]

Equivalent argmax formulation used on device (monotone transform, per row n):
    argmin_l dist  ==  argmax_l  T[n, l]
    T[n, l] = sum_d feats[n,d] * c[l,d]  +  cdvh_l + cdvl_l  +  u_n * (-w2c_l)
    u_n  = sqrt(|feats_n|^2 + 1) - sqrt(513)
    w2c_l = 0.5*|c_l|^2 - mean_l(0.5*|c_l|^2);  cdv_l = c[l,512] - sqrt(513)*w2c_l
    (cdv ships as bf16-exact hi + fp16 lo so every fp16 operand is small or exact)

Sharding: pure data-parallel over rows, N/8 = 32768 rows per NeuronCore.

Per-core dataflow (groups of GROUP=1024 rows = 8 subtiles of 128):
    DMA (gpsimd SWDGE): f32 -> fp16 in-flight, 4 consecutive rows per
        partition => 8 KiB contiguous HBM reads per descriptor
    DVE/ACT: square+accum -> q per row (split across both engines); ACT
        sqrt -> r; DVE writes u = r - sqrt(513) into Fgx[:, 2, :] whose
        rows 0..1 are memset to 1.0 (the "virtual dims" [1, 1, u])
    PE: 32x transpose 128x128 fp16 -> fT (d on partitions); 8x transpose
        [128, 3] Fgx -> [3, 128] mini-chunks placed at 32-aligned
        partition offsets so the rank-1 matmuls stay legal
    DVE: copy fT PSUM->SBUF (fp16, 2x mode), tiny [128, 2, 128] mini copy
    PE: T[n, l] accumulation with ft as the STATIONARY operand:
        32 matmuls (128-free) + 8 rank-1 matmuls [1,1,u]x[cdvh,cdvl,-w2c]
        into one [128, 8, 128] f32 PSUM tile
    DVE: tensor_reduce max over the 8 windows of 128 -> [128, 8] maxima,
         one max_index over all 1024 scores -> positions (u16) straight
         into the output accumulator; final single DMA; host: l = idx & 127

Scores stay exact fp32 from PSUM accumulation through reduce/max_index
(both read PSUM directly), so the only fp16 roundings are the feats cast,
u, and -w2c - same error budget as the 313us baseline (measured 1/262144
label mismatches).

Notes on this walrus build:
  - one sync wait per engine instruction: constants ship in single-DMA packs
    + PE/ACT warmup ops observe them; residual multi-wait instructions are
    split via same-engine NoOps (_split_multiwait).
  - no Pool-engine compute (walrus rejects it); Pool only issues the DMA.
"""

import os
import sys

import numpy as np

for _p in ("/opt/trn_rl_repo",):
    if _p not in sys.path and os.path.isdir(_p):
        sys.path.insert(0, _p)

import concourse.bass as bass
import concourse.mybir as mybir
import concourse.tile as tile
from concourse.bass_utils import run_bass_kernel_spmd

N, D, K = 262144, 512, 128
N_CORES = 8
ROWS_PER_CORE = N // N_CORES  # 32768
GROUP = 1024  # rows per group (8 subtiles of 128)
SQ_DVE_N = 1  # squares on DVE (rest on ACT)
DMA_CONSEC = 8  # consecutive feats rows per partition (4 => 8KB, 8 => 16KB reads)

F32 = mybir.dt.float32
F16 = mybir.dt.float16
U16 = mybir.dt.uint16
AF = mybir.ActivationFunctionType
ALU = mybir.AluOpType

# cpk16 layout (columns of one [128, 768] fp16 constant block, single DMA)
P16_IDENT = 0    # [128, 128] identity
P16_CT = 128     # [128, 512] cT: cT[p, 128k+l] = c[l, 128k+p]
P16_WCD = 640    # [3, 128] at partition offsets 0/32/64/96:
                 #   row0 cdvh, row1 cdvl, row2 -w2c (replicated 4x)
P16_W = 768

MBAR = float(np.sqrt(513.0))  # fixed shift for r; argmax-invariant


def _split_multiwait(nc):
    """Walrus (this build) allows one sync wait per engine instruction.

    Tile occasionally emits 2+ (data dep + buffer-slot release on another
    semaphore). Splitting is semantics-preserving: a same-engine NoOp placed
    immediately before the instruction carries the surplus waits; the engine
    executes in order, so all waits are still enforced before the instruction
    runs.
    """
    import bass_rust

    for fn in nc.m.functions:
        for blk in fn.blocks:
            out = []
            changed = False
            for inst in blk.instructions:
                si = getattr(inst, "sync_info", None)
                waits = list(si.on_wait) if si is not None else []
                if len(waits) > 1:
                    for w in waits[:-1]:
                        nop = mybir.InstNoOp(
                            name=nc.get_next_instruction_name(), ins=[], outs=[]
                        )
                        nop.engine = inst.engine
                        nop.sync_info = bass_rust.SyncInfo(
                            on_wait=[w], on_update=[]
                        )
                        out.append(nop)
                    inst.sync_info = bass_rust.SyncInfo(
                        on_wait=waits[-1:], on_update=list(si.on_update)
                    )
                    changed = True
                out.append(inst)
            if changed:
                blk.instructions = out


def build_core_program(rows=ROWS_PER_CORE, split_waits=True):
    """Bass program for one NeuronCore processing `rows` rows of feats."""
    assert rows % GROUP == 0
    ngroups = rows // GROUP

    nc = bass.Bass()
    feats_d = nc.declare_dram_parameter("feats", [rows, D], F32, isOutput=False)
    cpk16_d = nc.declare_dram_parameter("cpk16", [128, P16_W], F16, isOutput=False)
    cb32_d = nc.declare_dram_parameter("cb32", [128, 2], F32, isOutput=False)
    idx_d = nc.declare_dram_parameter("idx16", [128, ngroups, 8], U16, isOutput=True)

    with tile.TileContext(nc) as tc:
        with (
            tc.tile_pool(name="const", bufs=1) as constp,
            tc.tile_pool(name="fin", bufs=4) as finp,
            tc.tile_pool(name="fgx", bufs=2) as fgxp,
            tc.tile_pool(name="sq", bufs=4) as sqp,
            tc.tile_pool(name="qr", bufs=2) as qrp,
            tc.tile_pool(name="ftsb", bufs=4) as ftsbp,
            tc.tile_pool(name="ftm", bufs=2) as ftmsbp,
            tc.tile_pool(name="pm", bufs=2) as pmp,
            tc.tile_pool(name="outp", bufs=1) as outp,
            tc.tile_pool(name="ftp", bufs=3, space="PSUM") as ftpp,
            tc.tile_pool(name="tp", bufs=2, space="PSUM") as tpp,
            tc.tile_pool(name="ftmp", bufs=1, space="PSUM") as ftmpp,
        ):
            cpk16 = constp.tile([128, P16_W], F16)
            nc.sync.dma_start(cpk16[:], cpk16_d[:])
            ident16 = cpk16[:, P16_IDENT : P16_IDENT + 128]
            cT16 = cpk16[:, P16_CT : P16_CT + 512]
            cb32 = constp.tile([128, 2], F32)
            nc.sync.dma_start(cb32[:], cb32_d[:])
            bias1 = cb32[:, 0:1]
            bias0 = cb32[:, 1:2]
            idxacc = outp.tile([128, ngroups, 8], U16)

            # warmup: make PE/ACT observe the const DMA lanes via single-wait
            # instructions so later ops carry at most one new wait each; the
            # extra transposes also ramp the PE out of its low p-state while
            # the first feats DMA is in flight (full speed needs ~3us busy).
            warm_ps = ftmpp.tile([3, 8, 128], F16, tag="ftm_ps")
            for _ in range(24):
                nc.tensor.transpose(warm_ps[:, 0, :], ident16[:, 0:3], ident16)
            act_warm = qrp.tile([1, 1], F32, tag="act_warm")
            nc.scalar.copy(act_warm[:], cb32[0:1, 0:1])

            for g in range(ngroups):
                # feats DMA: DMA_CONSEC consecutive rows per partition
                #   4 => Fg[p, 4h+j, d] = feats[g*1024 + 512h + 4p + j] (8KB)
                #   8 => Fg[p, j, d]    = feats[g*1024 + 8p + j]       (16KB)
                Fg = finp.tile([128, 8, 512], F16)
                if DMA_CONSEC == 8:
                    src = feats_d[g * GROUP : (g + 1) * GROUP, :].rearrange(
                        "(p j) d -> p j d", p=128
                    )
                    nc.gpsimd.dma_start(Fg[:], src)
                else:
                    src = feats_d[g * GROUP : (g + 1) * GROUP, :].rearrange(
                        "(h p j) d -> p h j d", h=2, p=128
                    )
                    dst = Fg[:].rearrange("p (h j) d -> p h j d", h=2)
                    nc.gpsimd.dma_start(dst, src)

                # q_w = |row|^2 per partition; split DVE / ACT
                Q = qrp.tile([128, 8], F32, tag="Q")
                for w in range(8):
                    sq = sqp.tile([128, 512], F16, tag="sq")
                    if w < SQ_DVE_N:
                        nc.vector.scalar_tensor_tensor(
                            sq[:], Fg[:, w, :], 1.0, Fg[:, w, :],
                            ALU.mult, ALU.mult, accum_out=Q[:, w : w + 1],
                        )
                    else:
                        nc.scalar.activation(
                            sq[:], Fg[:, w, :], AF.Square,
                            bias=bias0, accum_out=Q[:, w : w + 1],
                        )
                R = qrp.tile([128, 8], F32, tag="R")
                nc.scalar.activation(R[:], Q[:], AF.Sqrt, bias=bias1, scale=1.0)

                # virtual dims [1, 1, u] per row (u = r - MBAR, fp16);
                # contiguous [128, 3, 8] layout keeps the writes cheap
                Fgx = fgxp.tile([128, 3, 8], F16)
                nc.vector.memset(Fgx[:, 0:2, :], 1.0)
                nc.vector.tensor_scalar_add(Fgx[:, 2, :], R[:], -MBAR)

                # T[n, l] accumulation: per k-chunk transpose Fg -> ft, then
                # 8 matmuls with ft as the stationary operand.
                T_ps = tpp.tile([128, 8, 128], F32)
                for k in range(4):
                    ft_ps = ftpp.tile([128, 1024], F16)
                    for j in range(8):
                        nc.tensor.transpose(
                            ft_ps[:, j * 128 : (j + 1) * 128],
                            Fg[:, j, k * 128 : (k + 1) * 128],
                            ident16,
                        )
                    ft = ftsbp.tile([128, 1024], F16)
                    nc.vector.tensor_copy(ft[:], ft_ps[:])
                    for j in range(8):
                        # PSUM zero regions are bank-granular (4 slices):
                        # start once per bank, the bank-mates ride on it.
                        nc.tensor.matmul(
                            T_ps[:, j, :],
                            ft[:, j * 128 : (j + 1) * 128],
                            cT16[:, k * 128 : (k + 1) * 128],
                            start=(k == 0 and j % 4 == 0),
                            stop=False,
                        )

                # rank-1 fixups: [128, 3] mini transposes (each j gets its
                # own PSUM free-range - zero regions ignore partitions), one
                # [3, 8, 128] copy on DVE, then 8 contract-3 matmuls vs wcd16.
                ftm_ps = ftmpp.tile([3, 8, 128], F16, tag="ftm_ps")
                for j in range(8):
                    nc.tensor.transpose(ftm_ps[:, j, :], Fgx[:, :, j], ident16)
                ftm = ftmsbp.tile([3, 8, 128], F16)
                nc.vector.tensor_copy(ftm[:], ftm_ps[:])
                for j in range(8):
                    nc.tensor.matmul(
                        T_ps[:, j, :],
                        ftm[:, j, :],
                        cpk16[0:3, P16_WCD : P16_WCD + 128],
                        start=False, stop=(j % 4 == 3),
                    )

                # argmax over l per row: window maxima + one global max_index,
                # both reading the f32 scores straight from PSUM
                pm8 = pmp.tile([128, 8], F32)
                nc.vector.tensor_reduce(
                    pm8[:], T_ps[:], mybir.AxisListType.X, ALU.max,
                    opt_input=False,
                )
                nc.vector.max_index(
                    idxacc[:, g, :], pm8[:],
                    T_ps[:].rearrange("p j l -> p (j l)"),
                )

            nc.sync.dma_start(idx_d[:], idxacc[:])
    if split_waits:
        _split_multiwait(nc)
    return nc


def make_const_inputs(initc, labelset):
    c = np.asarray(initc, dtype=np.float32)[np.asarray(labelset).astype(np.int64)]
    assert c.shape == (K, D + 1)
    w2 = 0.5 * np.sum(c.astype(np.float64) ** 2, axis=1)
    # Centering w2 and r shifts scores by per-row constants (argmax invariant)
    # while keeping the fp16-resident rank-1 operands small. The leftover
    # per-l constant cdv = cd - MBAR*w2c is large, so it ships as a
    # bf16-exact hi part (fp16-exact too) plus a small fp16 lo remainder.
    w2c = w2 - w2.mean()
    cdv = c[:, D].astype(np.float64) - MBAR * w2c
    cdv_hi = cdv.astype(np.float32)
    cdv_hi = (cdv_hi.view(np.uint32) & np.uint32(0xFFFF0000)).view(np.float32)
    cdv_lo = (cdv - cdv_hi.astype(np.float64)).astype(np.float32)
    cdv_hi16 = cdv_hi.astype(np.float16)
    assert np.array_equal(cdv_hi16.astype(np.float32), cdv_hi), "cdv_hi not fp16-exact"

    cpk16 = np.zeros((128, P16_W), np.float16)
    cpk16[:, P16_IDENT : P16_IDENT + 128] = np.eye(128, dtype=np.float16)
    for k in range(4):
        cpk16[:, P16_CT + k * 128 : P16_CT + (k + 1) * 128] = (
            c[:, k * 128 : (k + 1) * 128].T.astype(np.float16)
        )
    for q in range(3):
        cpk16[32 * q + 0, P16_WCD : P16_WCD + 128] = cdv_hi16
        cpk16[32 * q + 1, P16_WCD : P16_WCD + 128] = cdv_lo.astype(np.float16)
        cpk16[32 * q + 2, P16_WCD : P16_WCD + 128] = (-w2c).astype(np.float16)

    cb32 = np.zeros((128, 2), np.float32)
    cb32[:, 0] = 1.0
    return {"cpk16": cpk16, "cb32": cb32}


def decode_idx(idx16, labelset_np):
    """[128, ngroups, 8] u16 window-argmax positions -> labels in row order."""
    ngroups = idx16.shape[1]
    l = (idx16.astype(np.int64) & 127)  # [p, g, w]
    if DMA_CONSEC == 8:
        # row n = g*1024 + 8p + w
        l = np.transpose(l, (1, 0, 2)).reshape(-1)  # [g, p, w]
    else:
        # row n = g*1024 + (w//4)*512 + 4p + (w%4)
        l = l.reshape(128, ngroups, 2, 4)           # [p, g, h, j]
        l = np.transpose(l, (1, 2, 0, 3)).reshape(-1)  # [g, h, p, j]
    return np.asarray(labelset_np)[l]


def kernel(feats, initc, labelset):
    feats = np.asarray(feats, dtype=np.float32)
    labelset_np = np.asarray(labelset)
    consts = make_const_inputs(initc, labelset)

    nc = build_core_program(ROWS_PER_CORE)
    in_maps = []
    for core in range(N_CORES):
        shard = feats[core * ROWS_PER_CORE : (core + 1) * ROWS_PER_CORE]
        in_maps.append({"feats": np.ascontiguousarray(shard), **consts})
    res = run_bass_kernel_spmd(nc, in_maps, list(range(N_CORES)))

    preds = []
    for core in range(N_CORES):
        idx16 = np.asarray(res.results[core]["idx16"])  # [128, ngroups, 8]
        preds.append(decode_idx(idx16, labelset_np))
    return np.concatenate(preds)


# revision 27
# speedup vs baseline: 1.6148x; 1.0288x over previous
"""Trainium2 Bass kernel: nearest-centroid assignment (vq_codebook).

For each row of `feats` [N, 512]:
    f = normalize([feats_n, 1])            (L2, with appended ones column)
    pred_n = labelset[argmin_l ||f - c_l||]   with c = initc[labelset]  [128, 513]

Equivalent argmax formulation used on device (monotone transform, per row n):
    argmin_l dist  ==  argmax_l  T[n, l]
    T[n, l] = sum_d feats[n,d] * c[l,d]  +  cdvh_l + cdvl_l  +  u_n * (-w2c_l)
    u_n  = sqrt(|feats_n|^2 + 1) - sqrt(513)
    w2c_l = 0.5*|c_l|^2 - mean_l(0.5*|c_l|^2);  cdv_l = c[l,512] - sqrt(513)*w2c_l
    (cdv ships as bf16-exact hi + fp16 lo so every fp16 operand is small or exact)

Sharding: pure data-parallel over rows, N/8 = 32768 rows per NeuronCore.

Per-core dataflow (groups of GROUP=1024 rows = 8 subtiles of 128):
    DMA (gpsimd SWDGE): f32 -> fp16 in-flight, 4 consecutive rows per
        partition => 8 KiB contiguous HBM reads per descriptor
    DVE/ACT: square+accum -> q per row (split across both engines); ACT
        sqrt -> r; DVE writes u = r - sqrt(513) into Fgx[:, 2, :] whose
        rows 0..1 are memset to 1.0 (the "virtual dims" [1, 1, u])
    PE: 32x transpose 128x128 fp16 -> fT (d on partitions); 8x transpose
        [128, 3] Fgx -> [3, 128] mini-chunks placed at 32-aligned
        partition offsets so the rank-1 matmuls stay legal
    DVE: copy fT PSUM->SBUF (fp16, 2x mode), tiny [128, 2, 128] mini copy
    PE: T[n, l] accumulation with ft as the STATIONARY operand:
        32 matmuls (128-free) + 8 rank-1 matmuls [1,1,u]x[cdvh,cdvl,-w2c]
        into one [128, 8, 128] f32 PSUM tile
    DVE: tensor_reduce max over the 8 windows of 128 -> [128, 8] maxima,
         one max_index over all 1024 scores -> positions (u16) straight
         into the output accumulator; final single DMA; host: l = idx & 127

Scores stay exact fp32 from PSUM accumulation through reduce/max_index
(both read PSUM directly), so the only fp16 roundings are the feats cast,
u, and -w2c - same error budget as the 313us baseline (measured 1/262144
label mismatches).

Notes on this walrus build:
  - one sync wait per engine instruction: constants ship in single-DMA packs
    + PE/ACT warmup ops observe them; residual multi-wait instructions are
    split via same-engine NoOps (_split_multiwait).
  - no Pool-engine compute (walrus rejects it); Pool only issues the DMA.
"""

import os
import sys

import numpy as np

for _p in ("/opt/trn_rl_repo",):
    if _p not in sys.path and os.path.isdir(_p):
        sys.path.insert(0, _p)

import concourse.bass as bass
import concourse.mybir as mybir
import concourse.tile as tile
from concourse.bass_utils import run_bass_kernel_spmd

N, D, K = 262144, 512, 128
N_CORES = 8
ROWS_PER_CORE = N // N_CORES  # 32768
GROUP = 1024  # rows per group (8 subtiles of 128)
SQ_DVE_N = 1  # squares on DVE (rest on ACT)
DMA_CONSEC = 8  # consecutive feats rows per partition (4 => 8KB, 8 => 16KB reads)

F32 = mybir.dt.float32
F16 = mybir.dt.float16
U16 = mybir.dt.uint16
AF = mybir.ActivationFunctionType
ALU = mybir.AluOpType

# cpk16 layout (columns of one [128, 768] fp16 constant block, single DMA)
P16_IDENT = 0    # [128, 128] identity
P16_CT = 128     # [128, 512] cT: cT[p, 128k+l] = c[l, 128k+p]
P16_WCD = 640    # [3, 128] at partition offsets 0/32/64/96:
                 #   row0 cdvh, row1 cdvl, row2 -w2c (replicated 4x)
P16_W = 768

MBAR = float(np.sqrt(513.0))  # fixed shift for r; argmax-invariant


def _split_multiwait(nc):
    """Walrus (this build) allows one sync wait per engine instruction.

    Tile occasionally emits 2+ (data dep + buffer-slot release on another
    semaphore). Splitting is semantics-preserving: a same-engine NoOp placed
    immediately before the instruction carries the surplus waits; the engine
    executes in order, so all waits are still enforced before the instruction
    runs.
    """
    import bass_rust

    for fn in nc.m.functions:
        for blk in fn.blocks:
            out = []
            changed = False
            for inst in blk.instructions:
                si = getattr(inst, "sync_info", None)
                waits = list(si.on_wait) if si is not None else []
                if len(waits) > 1:
                    for w in waits[:-1]:
                        nop = mybir.InstNoOp(
                            name=nc.get_next_instruction_name(), ins=[], outs=[]
                        )
                        nop.engine = inst.engine
                        nop.sync_info = bass_rust.SyncInfo(
                            on_wait=[w], on_update=[]
                        )
                        out.append(nop)
                    inst.sync_info = bass_rust.SyncInfo(
                        on_wait=waits[-1:], on_update=list(si.on_update)
                    )
                    changed = True
                out.append(inst)
            if changed:
                blk.instructions = out


def build_core_program(rows=ROWS_PER_CORE, split_waits=True):
    """Bass program for one NeuronCore processing `rows` rows of feats."""
    assert rows % GROUP == 0
    ngroups = rows // GROUP

    nc = bass.Bass()
    feats_d = nc.declare_dram_parameter("feats", [rows, D], F32, isOutput=False)
    cpk16_d = nc.declare_dram_parameter("cpk16", [128, P16_W], F16, isOutput=False)
    cb32_d = nc.declare_dram_parameter("cb32", [128, 2], F32, isOutput=False)
    idx_d = nc.declare_dram_parameter("idx16", [128, ngroups, 8], U16, isOutput=True)

    with tile.TileContext(nc) as tc:
        with (
            tc.tile_pool(name="const", bufs=1) as constp,
            tc.tile_pool(name="fin", bufs=4) as finp,
            tc.tile_pool(name="fgx", bufs=2) as fgxp,
            tc.tile_pool(name="sq", bufs=4) as sqp,
            tc.tile_pool(name="qr", bufs=2) as qrp,
            tc.tile_pool(name="ftsb", bufs=4) as ftsbp,
            tc.tile_pool(name="ftm", bufs=2) as ftmsbp,
            tc.tile_pool(name="pm", bufs=2) as pmp,
            tc.tile_pool(name="outp", bufs=1) as outp,
            tc.tile_pool(name="ftp", bufs=3, space="PSUM") as ftpp,
            tc.tile_pool(name="tp", bufs=2, space="PSUM") as tpp,
            tc.tile_pool(name="ftmp", bufs=1, space="PSUM") as ftmpp,
        ):
            cpk16 = constp.tile([128, P16_W], F16)
            nc.sync.dma_start(cpk16[:], cpk16_d[:])
            ident16 = cpk16[:, P16_IDENT : P16_IDENT + 128]
            cT16 = cpk16[:, P16_CT : P16_CT + 512]
            cb32 = constp.tile([128, 2], F32)
            nc.sync.dma_start(cb32[:], cb32_d[:])
            bias1 = cb32[:, 0:1]
            bias0 = cb32[:, 1:2]
            idxacc = outp.tile([128, ngroups, 8], U16)

            # warmup: make PE/ACT observe the const DMA lanes via single-wait
            # instructions so later ops carry at most one new wait each; the
            # extra transposes also ramp the PE out of its low p-state while
            # the first feats DMA is in flight (full speed needs ~3us busy).
            warm_ps = ftmpp.tile([3, 8, 128], F16, tag="ftm_ps")
            for _ in range(24):
                nc.tensor.transpose(warm_ps[:, 0, :], ident16[:, 0:3], ident16)
            act_warm = qrp.tile([1, 1], F32, tag="act_warm")
            nc.scalar.copy(act_warm[:], cb32[0:1, 0:1])

            def _emit_argmax(T_done, g_done):
                # argmax over l per row: window maxima + one global
                # max_index, both reading the f32 scores straight from PSUM
                pm8 = pmp.tile([128, 8], F32)
                nc.vector.tensor_reduce(
                    pm8[:], T_done[:], mybir.AxisListType.X, ALU.max,
                    opt_input=False,
                )
                nc.vector.max_index(
                    idxacc[:, g_done, :], pm8[:],
                    T_done[:].rearrange("p j l -> p (j l)"),
                )

            prev = None
            for g in range(ngroups):
                # feats DMA: DMA_CONSEC consecutive rows per partition
                #   4 => Fg[p, 4h+j, d] = feats[g*1024 + 512h + 4p + j] (8KB)
                #   8 => Fg[p, j, d]    = feats[g*1024 + 8p + j]       (16KB)
                Fg = finp.tile([128, 8, 512], F16)
                if DMA_CONSEC == 8:
                    src = feats_d[g * GROUP : (g + 1) * GROUP, :].rearrange(
                        "(p j) d -> p j d", p=128
                    )
                    nc.gpsimd.dma_start(Fg[:], src)
                else:
                    src = feats_d[g * GROUP : (g + 1) * GROUP, :].rearrange(
                        "(h p j) d -> p h j d", h=2, p=128
                    )
                    dst = Fg[:].rearrange("p (h j) d -> p h j d", h=2)
                    nc.gpsimd.dma_start(dst, src)

                # q_w = |row|^2 per partition; split DVE / ACT
                Q = qrp.tile([128, 8], F32, tag="Q")
                for w in range(8):
                    sq = sqp.tile([128, 512], F16, tag="sq")
                    if w < SQ_DVE_N:
                        nc.vector.scalar_tensor_tensor(
                            sq[:], Fg[:, w, :], 1.0, Fg[:, w, :],
                            ALU.mult, ALU.mult, accum_out=Q[:, w : w + 1],
                        )
                    else:
                        nc.scalar.activation(
                            sq[:], Fg[:, w, :], AF.Square,
                            bias=bias0, accum_out=Q[:, w : w + 1],
                        )
                R = qrp.tile([128, 8], F32, tag="R")
                nc.scalar.activation(R[:], Q[:], AF.Sqrt, bias=bias1, scale=1.0)

                # virtual dims [1, 1, u] per row (u = r - MBAR, fp16);
                # contiguous [128, 3, 8] layout keeps the writes cheap.
                # The 1.0 rows survive buffer rotation: write them only on
                # the first pass through each of the 2 pool buffers.
                Fgx = fgxp.tile([128, 3, 8], F16)
                if g < 2:
                    nc.vector.memset(Fgx[:, 0:2, :], 1.0)
                nc.vector.tensor_scalar_add(Fgx[:, 2, :], R[:], -MBAR)

                # T[n, l] accumulation: per k-chunk transpose Fg -> ft, then
                # 8 matmuls with ft as the stationary operand.
                T_ps = tpp.tile([128, 8, 128], F32)
                for k in range(4):
                    ft_ps = ftpp.tile([128, 1024], F16)
                    for j in range(8):
                        nc.tensor.transpose(
                            ft_ps[:, j * 128 : (j + 1) * 128],
                            Fg[:, j, k * 128 : (k + 1) * 128],
                            ident16,
                        )
                    ft = ftsbp.tile([128, 1024], F16)
                    nc.vector.tensor_copy(ft[:], ft_ps[:])
                    if k == 0 and prev is not None:
                        # software-pipelined argmax of the PREVIOUS group:
                        # emitted here so the in-order DVE queue never parks
                        # on a not-yet-finished PE accumulation (its deps
                        # completed before this group's transposes began).
                        _emit_argmax(*prev)
                        prev = None
                    for j in range(8):
                        # PSUM zero regions are bank-granular (4 slices):
                        # start once per bank, the bank-mates ride on it.
                        nc.tensor.matmul(
                            T_ps[:, j, :],
                            ft[:, j * 128 : (j + 1) * 128],
                            cT16[:, k * 128 : (k + 1) * 128],
                            start=(k == 0 and j % 4 == 0),
                            stop=False,
                        )

                # rank-1 fixups: [128, 3] mini transposes (each j gets its
                # own PSUM free-range - zero regions ignore partitions), one
                # [3, 8, 128] copy on DVE, then 8 contract-3 matmuls vs wcd16.
                ftm_ps = ftmpp.tile([3, 8, 128], F16, tag="ftm_ps")
                for j in range(8):
                    nc.tensor.transpose(ftm_ps[:, j, :], Fgx[:, :, j], ident16)
                ftm = ftmsbp.tile([3, 8, 128], F16)
                nc.vector.tensor_copy(ftm[:], ftm_ps[:])
                for j in range(8):
                    nc.tensor.matmul(
                        T_ps[:, j, :],
                        ftm[:, j, :],
                        cpk16[0:3, P16_WCD : P16_WCD + 128],
                        start=False, stop=(j % 4 == 3),
                    )

                prev = (T_ps, g)
            _emit_argmax(*prev)

            nc.sync.dma_start(idx_d[:], idxacc[:])
    if split_waits:
        _split_multiwait(nc)
    return nc


def make_const_inputs(initc, labelset):
    c = np.asarray(initc, dtype=np.float32)[np.asarray(labelset).astype(np.int64)]
    assert c.shape == (K, D + 1)
    w2 = 0.5 * np.sum(c.astype(np.float64) ** 2, axis=1)
    # Centering w2 and r shifts scores by per-row constants (argmax invariant)
    # while keeping the fp16-resident rank-1 operands small. The leftover
    # per-l constant cdv = cd - MBAR*w2c is large, so it ships as a
    # bf16-exact hi part (fp16-exact too) plus a small fp16 lo remainder.
    w2c = w2 - w2.mean()
    cdv = c[:, D].astype(np.float64) - MBAR * w2c
    cdv_hi = cdv.astype(np.float32)
    cdv_hi = (cdv_hi.view(np.uint32) & np.uint32(0xFFFF0000)).view(np.float32)
    cdv_lo = (cdv - cdv_hi.astype(np.float64)).astype(np.float32)
    cdv_hi16 = cdv_hi.astype(np.float16)
    assert np.array_equal(cdv_hi16.astype(np.float32), cdv_hi), "cdv_hi not fp16-exact"

    cpk16 = np.zeros((128, P16_W), np.float16)
    cpk16[:, P16_IDENT : P16_IDENT + 128] = np.eye(128, dtype=np.float16)
    for k in range(4):
        cpk16[:, P16_CT + k * 128 : P16_CT + (k + 1) * 128] = (
            c[:, k * 128 : (k + 1) * 128].T.astype(np.float16)
        )
    for q in range(3):
        cpk16[32 * q + 0, P16_WCD : P16_WCD + 128] = cdv_hi16
        cpk16[32 * q + 1, P16_WCD : P16_WCD + 128] = cdv_lo.astype(np.float16)
        cpk16[32 * q + 2, P16_WCD : P16_WCD + 128] = (-w2c).astype(np.float16)

    cb32 = np.zeros((128, 2), np.float32)
    cb32[:, 0] = 1.0
    return {"cpk16": cpk16, "cb32": cb32}


def decode_idx(idx16, labelset_np):
    """[128, ngroups, 8] u16 window-argmax positions -> labels in row order."""
    ngroups = idx16.shape[1]
    l = (idx16.astype(np.int64) & 127)  # [p, g, w]
    if DMA_CONSEC == 8:
        # row n = g*1024 + 8p + w
        l = np.transpose(l, (1, 0, 2)).reshape(-1)  # [g, p, w]
    else:
        # row n = g*1024 + (w//4)*512 + 4p + (w%4)
        l = l.reshape(128, ngroups, 2, 4)           # [p, g, h, j]
        l = np.transpose(l, (1, 2, 0, 3)).reshape(-1)  # [g, h, p, j]
    return np.asarray(labelset_np)[l]


def kernel(feats, initc, labelset):
    feats = np.asarray(feats, dtype=np.float32)
    labelset_np = np.asarray(labelset)
    consts = make_const_inputs(initc, labelset)

    nc = build_core_program(ROWS_PER_CORE)
    in_maps = []
    for core in range(N_CORES):
        shard = feats[core * ROWS_PER_CORE : (core + 1) * ROWS_PER_CORE]
        in_maps.append({"feats": np.ascontiguousarray(shard), **consts})
    res = run_bass_kernel_spmd(nc, in_maps, list(range(N_CORES)))

    preds = []
    for core in range(N_CORES):
        idx16 = np.asarray(res.results[core]["idx16"])  # [128, ngroups, 8]
        preds.append(decode_idx(idx16, labelset_np))
    return np.concatenate(preds)
